# revision 5
# baseline (speedup 1.0000x reference)
"""DGCN+LSTM Trainium2 kernel (8 NeuronCores, data-parallel over agents).

Math restructuring vs the reference:
  y[t,n,k] = conv_w @ (x[t,A[t,n,k]] - x[t,n]) + conv_b ; feat = max_k y
is linear in the gathered rows, so with u[t] = conv_w @ x[t]^T:
  feat[t,n] = max_k u[t,A[t,n,k]] - u[t,n] + conv_b
The conv bias is folded into the LSTM bias (b_eff = b_ih + b_hh + w_ih@conv_b),
so the device computes featd = max_k u[A] - u[self] and a standard LSTM.

Per core (256 agents): PE computes u_t^T [64,2048] for 2 timesteps packed on
128 partitions; GPSIMD ap_gather pulls the 256*32 neighbor rows (+256 self
rows, so the program is core-id independent) along the free axis; DVE does the
K=32 max-reduce and the LSTM elementwise; ACT does gate activations.
"""

import sys

sys.path.insert(0, "/opt/trn_rl_repo")

import numpy as np
import ml_dtypes

import concourse.bass as bass
import concourse.mybir as mybir
import concourse.tile as tile
from concourse import bacc
from concourse.bass_utils import run_bass_kernel_spmd

N, T, K, C, O = 2048, 20, 32, 32, 64
NCORES = 8
NS = N // NCORES          # agents per core
PAIRS = T // 2            # timestep pairs
NIDX = NS * K + NS        # gather indices per timestep (+self rows)
F32 = mybir.dt.float32
BF16 = mybir.dt.bfloat16
I16 = mybir.dt.int16
BF = ml_dtypes.bfloat16

_CACHE = {}


def _build():
    nc = bacc.Bacc("TRN2", target_bir_lowering=False, debug=False,
                   num_devices=NCORES)

    xw_d = nc.dram_tensor("xw", [64, PAIRS, N], BF16, kind="ExternalInput")
    lhsu_d = nc.dram_tensor("lhsu", [64, 128], BF16, kind="ExternalInput")
    idx_d = nc.dram_tensor("idx", [128, PAIRS, NIDX // 16], I16,
                           kind="ExternalInput")
    wih_d = nc.dram_tensor("wih", [128, 4, 64], F32, kind="ExternalInput")
    whh_d = nc.dram_tensor("whh", [65, 4, 64], F32, kind="ExternalInput")
    oh_d = nc.dram_tensor("oh", [T, 64, NS], F32, kind="ExternalOutput")
    oc_d = nc.dram_tensor("oc", [64, NS], F32, kind="ExternalOutput")

    with tile.TileContext(nc) as tc:
        with (
            tc.tile_pool(name="const", bufs=1) as const,
            tc.tile_pool(name="upool", bufs=2) as upool,
            tc.tile_pool(name="gpool", bufs=2) as gpool,
            tc.tile_pool(name="fpool", bufs=2) as fpool,
            tc.tile_pool(name="lpool", bufs=2) as lpool,
            tc.tile_pool(name="state", bufs=1) as state,
            tc.tile_pool(name="psu", bufs=1, space="PSUM") as psu,
            tc.tile_pool(name="psg", bufs=2, space="PSUM") as psg,
        ):
            x_sb = const.tile([64, PAIRS, N], BF16)
            nc.sync.dma_start(x_sb[:], xw_d[:])
            lhsu = const.tile([64, 128], BF16)
            nc.sync.dma_start(lhsu[:], lhsu_d[:])
            idx_sb = const.tile([128, PAIRS, NIDX // 16], I16)
            nc.sync.dma_start(idx_sb[:], idx_d[:])
            wih = const.tile([128, 4, 64], F32)
            nc.sync.dma_start(wih[:], wih_d[:])
            whh = const.tile([65, 4, 64], F32)
            nc.sync.dma_start(whh[:], whh_d[:])

            h_aug = state.tile([65, NS], F32)   # rows 0:64 = h^T, row 64 = 1
            c_st = state.tile([64, NS], F32)
            nc.vector.memset(h_aug[0:64, :], 0.0)
            nc.vector.memset(h_aug[64:65, :], 1.0)
            nc.vector.memset(c_st[:], 0.0)

            for p in range(PAIRS):
                # u for t=2p (partitions 0:64) and t=2p+1 (64:128)
                u_ps = psu.tile([128, N], F32)
                for j in range(4):
                    nc.tensor.matmul(
                        u_ps[:, 512 * j:512 * (j + 1)],
                        lhsu[:],
                        x_sb[:, p, 512 * j:512 * (j + 1)],
                        start=True, stop=True,
                    )
                u_sb = upool.tile([128, N, 1], F32)
                nc.scalar.activation(u_sb[:, :, 0], u_ps[:],
                                     mybir.ActivationFunctionType.Copy)

                g2 = gpool.tile([128, NIDX, 1], F32)
                nc.gpsimd.ap_gather(
                    g2[:], u_sb[:], idx_sb[:, p, :],
                    channels=128, num_elems=N, d=1, num_idxs=NIDX,
                )

                mx = fpool.tile([128, NS], F32)
                nc.vector.tensor_reduce(
                    mx[:],
                    g2[:, 0:NS * K, 0].rearrange("c (n k) -> c n k", k=K),
                    axis=mybir.AxisListType.X,
                    op=mybir.AluOpType.max,
                )
                feat = fpool.tile([128, NS], F32)
                nc.vector.tensor_sub(feat[:], mx[:], g2[:, NS * K:NIDX, 0])

                for a in range(2):
                    t = 2 * p + a
                    ps_g = psg.tile([64, 1024], F32)
                    for j in range(4):
                        nc.tensor.matmul(
                            ps_g[:, 256 * j:256 * (j + 1)],
                            wih[64 * a:64 * (a + 1), j, :],
                            feat[64 * a:64 * (a + 1), :],
                            start=True, stop=False,
                        )
                        nc.tensor.matmul(
                            ps_g[:, 256 * j:256 * (j + 1)],
                            whh[:, j, :],
                            h_aug[:],
                            start=False, stop=True,
                        )
                    # gate order in chunks: i, f, o, g
                    sifo = lpool.tile([64, 768], F32)
                    nc.scalar.activation(sifo[:], ps_g[:, 0:768],
                                         mybir.ActivationFunctionType.Sigmoid)
                    tg = lpool.tile([64, NS], F32)
                    nc.scalar.activation(tg[:], ps_g[:, 768:1024],
                                         mybir.ActivationFunctionType.Tanh)
                    t1 = lpool.tile([64, NS], F32)
                    nc.vector.tensor_mul(t1[:], sifo[:, 256:512], c_st[:])
                    t2 = lpool.tile([64, NS], F32)
                    nc.vector.tensor_mul(t2[:], sifo[:, 0:256], tg[:])
                    nc.vector.tensor_add(c_st[:], t1[:], t2[:])
                    thc = lpool.tile([64, NS], F32)
                    nc.scalar.activation(thc[:], c_st[:],
                                         mybir.ActivationFunctionType.Tanh)
                    nc.vector.tensor_mul(h_aug[0:64, :], sifo[:, 512:768],
                                         thc[:])
                    nc.sync.dma_start(oh_d.ap()[t], h_aug[0:64, :])

            nc.sync.dma_start(oc_d.ap()[:], c_st[:])

    nc.compile()
    return nc


def _prep_inputs(x, A, conv_w, conv_b, w_ih, w_hh, b_ih, b_hh):
    x = np.asarray(x, np.float32)
    A = np.asarray(A)
    conv_w = np.asarray(conv_w, np.float32)
    conv_b = np.asarray(conv_b, np.float32)
    w_ih = np.asarray(w_ih, np.float32)
    w_hh = np.asarray(w_hh, np.float32)
    b_ih = np.asarray(b_ih, np.float32)
    b_hh = np.asarray(b_hh, np.float32)

    # x_sb[32*a + c, p, n] = x[n, 2p+a, c]
    xp = x.transpose(1, 2, 0)                      # [T, C, N]
    xw = (xp.reshape(PAIRS, 2, C, N)
            .transpose(1, 2, 0, 3)
            .reshape(64, PAIRS, N)).astype(BF)

    lhsu = np.zeros((64, 128), np.float32)
    lhsu[0:32, 0:64] = conv_w.T
    lhsu[32:64, 64:128] = conv_w.T
    lhsu = lhsu.astype(BF)

    # per-core gather indices, wrapped per 16 partitions, replicated over the
    # 4 groups serving each timestep; groups 0-3 -> t=2p, 4-7 -> t=2p+1
    idx_all = np.empty((NCORES, 128, PAIRS, NIDX // 16), np.int16)
    for cid in range(NCORES):
        base = cid * NS
        for p in range(PAIRS):
            for a in range(2):
                t = 2 * p + a
                flat = np.concatenate([
                    A[t, base:base + NS, :].reshape(-1),
                    np.arange(base, base + NS),
                ]).astype(np.int16)                         # [NIDX]
                wrapped = flat.reshape(NIDX // 16, 16).T    # [16, NIDX//16]
                idx_all[cid, 64 * a:64 * (a + 1), p, :] = np.tile(wrapped,
                                                                  (4, 1))

    jorder = [0, 1, 3, 2]   # our chunk order i,f,o,g from torch i,f,g,o
    bias_eff = b_ih + b_hh + w_ih @ conv_b
    wih_l = np.empty((128, 4, 64), np.float32)
    whh_l = np.empty((65, 4, 64), np.float32)
    for j, jo in enumerate(jorder):
        wih_l[0:64, j, :] = w_ih[64 * jo:64 * (jo + 1), :].T
        wih_l[64:128, j, :] = wih_l[0:64, j, :]
        whh_l[0:64, j, :] = w_hh[64 * jo:64 * (jo + 1), :].T
        whh_l[64, j, :] = bias_eff[64 * jo:64 * (jo + 1)]

    in_maps = []
    for cid in range(NCORES):
        in_maps.append({
            "xw": xw,
            "lhsu": lhsu,
            "idx": idx_all[cid],
            "wih": wih_l,
            "whh": whh_l,
        })
    return in_maps


def kernel(x, A, conv_w, conv_b, w_ih, w_hh, b_ih, b_hh, _bench=None):
    if "nc" not in _CACHE:
        _CACHE["nc"] = _build()
    nc = _CACHE["nc"]

    in_maps = _prep_inputs(x, A, conv_w, conv_b, w_ih, w_hh, b_ih, b_hh)
    res = run_bass_kernel_spmd(nc, in_maps, core_ids=list(range(NCORES)),
                               **(_bench or {}))
    if _bench is not None:
        _CACHE["last_results"] = res

    out = np.empty((N, T, O), np.float32)
    cN = np.empty((N, 1, O), np.float32)
    for cid in range(NCORES):
        r = res.results[cid]
        base = cid * NS
        out[base:base + NS] = r["oh"].transpose(2, 0, 1)    # [NS, T, O]
        cN[base:base + NS, 0, :] = r["oc"].T
    hN = out[:, T - 1:T, :].copy()
    return out, hN, cN


# revision 14
# speedup vs baseline: 1.7060x; 1.7060x over previous
"""DGCN+LSTM Trainium2 kernel (8 NeuronCores, data-parallel over agents).

Math restructuring vs the reference:
  y[t,n,k] = conv_w @ (x[t,A[t,n,k]] - x[t,n]) + conv_b ; feat = max_k y
is linear in the gathered rows, so with u[t] = conv_w @ x[t]^T:
  feat[t,n] = max_k u[t,A[t,n,k]] - u[t,n] + conv_b
The conv bias is folded into the LSTM bias (b_eff = b_ih + b_hh + w_ih@conv_b)
and the self-rows u[t,n] are appended to the gather index list, so the NEFF is
identical on all cores (pure SPMD; per-core behavior comes only from inputs).

Per core (256 agents), per timestep:
  PE   : u[t] rows agent-major [128p, 16 stripes, 64ch] -> DRAM rows (bf16)
  Q7   : dma_gather (HBM src, 256B granule = wanted row + next row) in
         <=1920-index chunks (SWDGE ring is 128 descriptors/engine);
         k-major index order puts row i at partition i%128, so the K=32
         max tree is contiguous block halves at full 128-lane width
  DVE  : log2(K) tensor_max tree in bf16 (2x mode), self-subtract
  PE   : 2 transposes feat [n%128, nb*ch] -> [ch, n] for the LSTM
  PE/ACT/DVE: fused-bias LSTM step
"""

import sys

sys.path.insert(0, "/opt/trn_rl_repo")

import numpy as np
import ml_dtypes

import concourse.bass as bass
import concourse.mybir as mybir
import concourse.tile as tile
from concourse import bacc
from concourse.bass_utils import run_bass_kernel_spmd

N, T, K, C, O = 2048, 20, 32, 32, 64
NCORES = 8
NS = N // NCORES          # agents per core
PAIRS = T // 2
NIDX = NS * K + NS        # 8448 gather indices per timestep (neighbors+self)
NNB = NS * K
CHUNKS = (1024,) * 8 + (256,)   # SWDGE ring: 64 data descs/engine = 1024 idx
NROW = 2064               # u rows per timestep in DRAM (2048 + pad)
F32 = mybir.dt.float32
BF16 = mybir.dt.bfloat16
I16 = mybir.dt.int16
BF = ml_dtypes.bfloat16

_CACHE = {}


def _build():
    nc = bacc.Bacc("TRN2", target_bir_lowering=False, debug=False,
                   num_devices=NCORES)

    # xd[p, c, a, n] = x[n, 2p+a, c]
    xd = nc.dram_tensor("xd", [PAIRS, C, 2, N], BF16, kind="ExternalInput")
    cw_d = nc.dram_tensor("cw", [C, O], BF16, kind="ExternalInput")
    idx_d = nc.dram_tensor("idx", [128, T, NIDX // 16], I16,
                           kind="ExternalInput")
    wih_d = nc.dram_tensor("wih", [64, 4, 64], BF16, kind="ExternalInput")
    whh_d = nc.dram_tensor("whh", [65, 4, 64], BF16, kind="ExternalInput")
    id_d = nc.dram_tensor("ident", [128, 128], BF16, kind="ExternalInput")
    oh_d = nc.dram_tensor("oh", [T, 64, NS], BF16, kind="ExternalOutput")
    oc_d = nc.dram_tensor("oc", [64, NS], BF16, kind="ExternalOutput")
    u_dram = nc.dram_tensor("u_scratch", [T, NROW, 128], BF16,
                            kind="Internal")

    with tile.TileContext(nc) as tc:
        with (
            tc.tile_pool(name="const", bufs=1) as const,
            tc.tile_pool(name="xpool", bufs=2) as xpool,
            tc.tile_pool(name="upool", bufs=2) as upool,
            tc.tile_pool(name="gpool", bufs=2) as gpool,
            tc.tile_pool(name="tpool", bufs=1) as tpool,
            tc.tile_pool(name="fpool", bufs=2) as fpool,
            tc.tile_pool(name="lpool", bufs=2) as lpool,
            tc.tile_pool(name="state", bufs=1) as state,
            tc.tile_pool(name="psu", bufs=2, space="PSUM") as psu,
            tc.tile_pool(name="psf", bufs=2, space="PSUM") as psf,
            tc.tile_pool(name="psg", bufs=1, space="PSUM") as psg,
        ):
            cw = const.tile([C, O], BF16)
            nc.sync.dma_start(cw[:], cw_d[:])
            idx_sb = const.tile([128, T, NIDX // 16], I16)
            nc.sync.dma_start(idx_sb[:], idx_d[:])
            wih = const.tile([64, 4, 64], BF16)
            nc.sync.dma_start(wih[:], wih_d[:])
            whh = const.tile([65, 4, 64], BF16)
            nc.sync.dma_start(whh[:], whh_d[:])
            ident = const.tile([128, 128], BF16)
            nc.sync.dma_start(ident[:], id_d[:])

            h_aug = state.tile([65, NS], BF16)  # rows 0:64 = h^T, row 64 = 1
            c_st = state.tile([64, NS], BF16)
            nc.vector.memset(h_aug[0:64, :], 0.0)
            nc.vector.memset(h_aug[64:65, :], 1.0)
            nc.vector.memset(c_st[:], 0.0)

            for p in range(PAIRS):
                xt = xpool.tile([C, 2, N], BF16)
                nc.sync.dma_start(xt[:], xd.ap()[p])

                # u rows, agent-major: u2[pp, 16a+j, ch] = u[2p+a, 128j+pp, ch]
                u2 = upool.tile([128, 32, O], BF16)
                for a in range(2):
                    ups = psu.tile([128, 16, O], F32)
                    for j in range(16):
                        nc.tensor.matmul(
                            ups[:, j, :],
                            xt[:, a, 128 * j:128 * (j + 1)],
                            cw[:],
                            start=True, stop=True,
                        )
                    nc.scalar.activation(u2[:, 16 * a:16 * (a + 1), :],
                                         ups[:],
                                         mybir.ActivationFunctionType.Copy)
                    # u rows to DRAM: row 128j+pp of timestep t
                    nc.sync.dma_start(
                        u_dram.ap()[2 * p + a, 0:2048, 0:O]
                        .rearrange("(j pp) e -> pp j e", pp=128),
                        u2[:, 16 * a:16 * (a + 1), :],
                    )

                for a in range(2):
                    t = 2 * p + a
                    # k-major gather: flat index i = k*256+n -> row A[t,n,k],
                    # partition i%128 = n%128, block i//128 = 2k + n//128.
                    # self rows appended at blocks 64, 65. 256B granule reads
                    # the wanted bf16 row plus the next row (cols 64:128).
                    g2 = gpool.tile([128, 66, 128], BF16)
                    c0 = 0
                    for chk in CHUNKS:
                        nc.gpsimd.dma_gather(
                            g2[:, c0 // 128:(c0 + chk) // 128, :],
                            u_dram.ap()[t],
                            idx_sb[:, t, c0 // 16:(c0 + chk) // 16],
                            num_idxs=chk, num_idxs_reg=chk,
                            elem_size=128, elem_step=128,
                        )
                        c0 += chk

                    # max over K=32: tree levels are contiguous block halves
                    l1 = tpool.tile([128, 16, 2, O], BF16)
                    nc.vector.tensor_max(
                        l1[:],
                        g2[:, 0:32, 0:O].rearrange("p (k b) e -> p k b e",
                                                   b=2),
                        g2[:, 32:64, 0:O].rearrange("p (k b) e -> p k b e",
                                                    b=2),
                    )
                    l2 = tpool.tile([128, 8, 2, O], BF16)
                    nc.vector.tensor_max(l2[:], l1[:, 0:8], l1[:, 8:16])
                    l3 = tpool.tile([128, 4, 2, O], BF16)
                    nc.vector.tensor_max(l3[:], l2[:, 0:4], l2[:, 4:8])
                    l4 = tpool.tile([128, 2, 2, O], BF16)
                    nc.vector.tensor_max(l4[:], l3[:, 0:2], l3[:, 2:4])
                    l5 = tpool.tile([128, 1, 2, O], BF16)
                    nc.vector.tensor_max(l5[:], l4[:, 0:1], l4[:, 1:2])
                    # featd[p, nb, ch] = max - self; self rows at blocks 64:66
                    featd = fpool.tile([128, 2, O], BF16)
                    nc.vector.tensor_sub(featd[:], l5[:, 0],
                                         g2[:, 64:66, 0:O])

                    # transpose to LSTM layout [ch, n]: two PE transposes
                    ps_f = psf.tile([64, NS], BF16)
                    nc.tensor.transpose(ps_f[:, 0:128], featd[:, 0, :],
                                        ident[:])
                    nc.tensor.transpose(ps_f[:, 128:256], featd[:, 1, :],
                                        ident[:])
                    featT = fpool.tile([64, NS], BF16)
                    nc.scalar.activation(featT[:], ps_f[:],
                                         mybir.ActivationFunctionType.Copy)

                    ps_g = psg.tile([64, 1024], F32)
                    for j in range(4):
                        nc.tensor.matmul(
                            ps_g[:, 256 * j:256 * (j + 1)],
                            wih[:, j, :],
                            featT[:],
                            start=True, stop=False,
                        )
                        nc.tensor.matmul(
                            ps_g[:, 256 * j:256 * (j + 1)],
                            whh[:, j, :],
                            h_aug[:],
                            start=False, stop=True,
                        )
                    # gate chunk order: i, f, o, g
                    sifo = lpool.tile([64, 768], BF16)
                    nc.scalar.activation(sifo[:], ps_g[:, 0:768],
                                         mybir.ActivationFunctionType.Sigmoid)
                    tg = lpool.tile([64, NS], BF16)
                    nc.scalar.activation(tg[:], ps_g[:, 768:1024],
                                         mybir.ActivationFunctionType.Tanh)
                    t1 = lpool.tile([64, NS], BF16)
                    nc.vector.tensor_mul(t1[:], sifo[:, 256:512], c_st[:])
                    t2 = lpool.tile([64, NS], BF16)
                    nc.vector.tensor_mul(t2[:], sifo[:, 0:256], tg[:])
                    nc.vector.tensor_add(c_st[:], t1[:], t2[:])
                    thc = lpool.tile([64, NS], BF16)
                    nc.scalar.activation(thc[:], c_st[:],
                                         mybir.ActivationFunctionType.Tanh)
                    nc.vector.tensor_mul(h_aug[0:64, :], sifo[:, 512:768],
                                         thc[:])
                    nc.sync.dma_start(oh_d.ap()[t], h_aug[0:64, :])

            nc.sync.dma_start(oc_d.ap()[:], c_st[:])

    nc.compile()
    return nc


def _prep_inputs(x, A, conv_w, conv_b, w_ih, w_hh, b_ih, b_hh):
    x = np.asarray(x, np.float32)
    A = np.asarray(A)
    conv_w = np.asarray(conv_w, np.float32)
    conv_b = np.asarray(conv_b, np.float32)
    w_ih = np.asarray(w_ih, np.float32)
    w_hh = np.asarray(w_hh, np.float32)
    b_ih = np.asarray(b_ih, np.float32)
    b_hh = np.asarray(b_hh, np.float32)

    xd = np.ascontiguousarray(
        x.transpose(1, 2, 0).reshape(PAIRS, 2, C, N).transpose(0, 2, 1, 3)
    ).astype(BF)                                            # [10, 32, 2, 2048]
    cw = np.ascontiguousarray(conv_w.T).astype(BF)          # [32, 64]

    # k-major flat index list [nbrs (k,n) | self], wrapped per 16 partitions
    # per chunk, replicated across the 8 GPSIMD cores' partition groups
    idx_all = np.empty((NCORES, 128, T, NIDX // 16), np.int16)
    ar = np.arange(NS, dtype=np.int16)
    for cid in range(NCORES):
        base = cid * NS
        for t in range(T):
            flat = np.concatenate([
                np.ascontiguousarray(A[t, base:base + NS, :].T).reshape(-1),
                base + ar,
            ]).astype(np.int16)                             # [NIDX] k-major
            w = np.empty((16, NIDX // 16), np.int16)
            c0 = 0
            for chk in CHUNKS:
                w[:, c0 // 16:(c0 + chk) // 16] = \
                    flat[c0:c0 + chk].reshape(chk // 16, 16).T
                c0 += chk
            idx_all[cid, :, t, :] = np.tile(w, (8, 1))

    jorder = [0, 1, 3, 2]   # chunk order i,f,o,g from torch's i,f,g,o
    bias_eff = b_ih + b_hh + w_ih @ conv_b
    wih_l = np.empty((64, 4, 64), np.float32)
    whh_l = np.empty((65, 4, 64), np.float32)
    for j, jo in enumerate(jorder):
        wih_l[:, j, :] = w_ih[64 * jo:64 * (jo + 1), :].T
        whh_l[0:64, j, :] = w_hh[64 * jo:64 * (jo + 1), :].T
        whh_l[64, j, :] = bias_eff[64 * jo:64 * (jo + 1)]
    wih_l = wih_l.astype(BF)
    whh_l = whh_l.astype(BF)
    ident = np.eye(128, dtype=np.float32).astype(BF)

    in_maps = []
    for cid in range(NCORES):
        in_maps.append({
            "xd": xd,
            "cw": cw,
            "idx": idx_all[cid],
            "wih": wih_l,
            "whh": whh_l,
            "ident": ident,
        })
    return in_maps


def kernel(x, A, conv_w, conv_b, w_ih, w_hh, b_ih, b_hh, _bench=None):
    if "nc" not in _CACHE:
        _CACHE["nc"] = _build()
    nc = _CACHE["nc"]

    in_maps = _prep_inputs(x, A, conv_w, conv_b, w_ih, w_hh, b_ih, b_hh)
    res = run_bass_kernel_spmd(nc, in_maps, core_ids=list(range(NCORES)),
                               **(_bench or {}))
    if _bench is not None:
        _CACHE["last_results"] = res

    out = np.empty((N, T, O), np.float32)
    cN = np.empty((N, 1, O), np.float32)
    for cid in range(NCORES):
        r = res.results[cid]
        base = cid * NS
        out[base:base + NS] = r["oh"].astype(np.float32).transpose(2, 0, 1)
        cN[base:base + NS, 0, :] = r["oc"].astype(np.float32).T
    hN = out[:, T - 1:T, :].copy()
    return out, hN, cN


# revision 16
# speedup vs baseline: 1.7614x; 1.0325x over previous
"""DGCN+LSTM Trainium2 kernel (8 NeuronCores, data-parallel over agents).

Math restructuring vs the reference:
  y[t,n,k] = conv_w @ (x[t,A[t,n,k]] - x[t,n]) + conv_b ; feat = max_k y
is linear in the gathered rows, so with u[t] = conv_w @ x[t]^T:
  feat[t,n] = max_k u[t,A[t,n,k]] - u[t,n] + conv_b
The conv bias is folded into the LSTM bias (b_eff = b_ih + b_hh + w_ih@conv_b)
and the self-rows u[t,n] are appended to the gather index list, so the NEFF is
identical on all cores (pure SPMD; per-core behavior comes only from inputs).

Per core (256 agents), per timestep:
  PE   : u[t] rows agent-major [128p, 16 stripes, 64ch] -> DRAM rows (bf16)
  Q7   : dma_gather (HBM src, 256B granule = wanted row + next row) in
         <=1920-index chunks (SWDGE ring is 128 descriptors/engine);
         k-major index order puts row i at partition i%128, so the K=32
         max tree is contiguous block halves at full 128-lane width
  DVE  : log2(K) tensor_max tree in bf16 (2x mode), self-subtract
  PE   : 2 transposes feat [n%128, nb*ch] -> [ch, n] for the LSTM
  PE/ACT/DVE: fused-bias LSTM step
"""

import sys

sys.path.insert(0, "/opt/trn_rl_repo")

import numpy as np
import ml_dtypes

import concourse.bass as bass
import concourse.mybir as mybir
import concourse.tile as tile
from concourse import bacc
from concourse.bass_utils import run_bass_kernel_spmd

N, T, K, C, O = 2048, 20, 32, 32, 64
NCORES = 8
NS = N // NCORES          # agents per core
PAIRS = T // 2
NIDX = NS * K             # 8192 gather indices per timestep (neighbors)
NNB = NS * K
CHUNKS = (1024,) * 8      # SWDGE ring: 64 data descriptors/engine = 1024 idx
NROW = 2064               # u rows per timestep in DRAM (2048 + pad)
F32 = mybir.dt.float32
BF16 = mybir.dt.bfloat16
I16 = mybir.dt.int16
BF = ml_dtypes.bfloat16

_CACHE = {}


def _build():
    nc = bacc.Bacc("TRN2", target_bir_lowering=False, debug=False,
                   num_devices=NCORES)

    # xd[p, c, a, n] = x[n, 2p+a, c]
    xd = nc.dram_tensor("xd", [PAIRS, C, 2, N], BF16, kind="ExternalInput")
    cw_d = nc.dram_tensor("cw", [C, O], BF16, kind="ExternalInput")
    idx_d = nc.dram_tensor("idx", [128, T, NIDX // 16], I16,
                           kind="ExternalInput")
    wih_d = nc.dram_tensor("wih", [64, 4, 64], BF16, kind="ExternalInput")
    whh_d = nc.dram_tensor("whh", [65, 4, 64], BF16, kind="ExternalInput")
    id_d = nc.dram_tensor("ident", [128, 128], BF16, kind="ExternalInput")
    xm_d = nc.dram_tensor("xm", [PAIRS, C, 2, NS], BF16, kind="ExternalInput")
    oh_d = nc.dram_tensor("oh", [T, 64, NS], BF16, kind="ExternalOutput")
    oc_d = nc.dram_tensor("oc", [64, NS], BF16, kind="ExternalOutput")
    u_dram = nc.dram_tensor("u_scratch", [T, NROW, 128], BF16,
                            kind="Internal")

    with tile.TileContext(nc) as tc:
        with (
            tc.tile_pool(name="const", bufs=1) as const,
            tc.tile_pool(name="xpool", bufs=2) as xpool,
            tc.tile_pool(name="upool", bufs=2) as upool,
            tc.tile_pool(name="gpool", bufs=2) as gpool,
            tc.tile_pool(name="tpool", bufs=1) as tpool,
            tc.tile_pool(name="fpool", bufs=2) as fpool,
            tc.tile_pool(name="lpool", bufs=2) as lpool,
            tc.tile_pool(name="state", bufs=1) as state,
            tc.tile_pool(name="psu", bufs=2, space="PSUM") as psu,
            tc.tile_pool(name="psf", bufs=1, space="PSUM") as psf,
            tc.tile_pool(name="psg", bufs=1, space="PSUM") as psg,
        ):
            cw = const.tile([C, O], BF16)
            nc.sync.dma_start(cw[:], cw_d[:])
            idx_sb = const.tile([128, T, NIDX // 16], I16)
            nc.sync.dma_start(idx_sb[:], idx_d[:])
            wih = const.tile([64, 4, 64], BF16)
            nc.sync.dma_start(wih[:], wih_d[:])
            whh = const.tile([65, 4, 64], BF16)
            nc.sync.dma_start(whh[:], whh_d[:])
            ident = const.tile([128, 128], BF16)
            nc.sync.dma_start(ident[:], id_d[:])

            h_aug = state.tile([65, NS], BF16)  # rows 0:64 = h^T, row 64 = 1
            c_st = state.tile([64, NS], BF16)
            nc.vector.memset(h_aug[0:64, :], 0.0)
            nc.vector.memset(h_aug[64:65, :], 1.0)
            nc.vector.memset(c_st[:], 0.0)

            for p in range(PAIRS):
                xt = xpool.tile([C, 2, N], BF16)
                nc.sync.dma_start(xt[:], xd.ap()[p])
                xm = xpool.tile([C, 2, NS], BF16)
                nc.sync.dma_start(xm[:], xm_d.ap()[p])

                # u rows, agent-major: u2[pp, 16a+j, ch] = u[2p+a, 128j+pp, ch]
                u2 = upool.tile([128, 32, O], BF16)
                for a in range(2):
                    ups = psu.tile([128, 16, O], F32)
                    for j in range(16):
                        nc.tensor.matmul(
                            ups[:, j, :],
                            xt[:, a, 128 * j:128 * (j + 1)],
                            cw[:],
                            start=True, stop=True,
                        )
                    nc.scalar.activation(u2[:, 16 * a:16 * (a + 1), :],
                                         ups[:],
                                         mybir.ActivationFunctionType.Copy)
                    # u rows to DRAM: row 128j+pp of timestep t
                    nc.sync.dma_start(
                        u_dram.ap()[2 * p + a, 0:2048, 0:O]
                        .rearrange("(j pp) e -> pp j e", pp=128),
                        u2[:, 16 * a:16 * (a + 1), :],
                    )

                for a in range(2):
                    t = 2 * p + a
                    # k-major gather: flat index i = k*256+n -> row A[t,n,k],
                    # partition i%128 = n%128, block i//128 = 2k + n//128.
                    # self rows appended at blocks 64, 65. 256B granule reads
                    # the wanted bf16 row plus the next row (cols 64:128).
                    g2 = gpool.tile([128, 64, 128], BF16)
                    c0 = 0
                    for chk in CHUNKS:
                        nc.gpsimd.dma_gather(
                            g2[:, c0 // 128:(c0 + chk) // 128, :],
                            u_dram.ap()[t],
                            idx_sb[:, t, c0 // 16:(c0 + chk) // 16],
                            num_idxs=chk, num_idxs_reg=chk,
                            elem_size=128, elem_step=128,
                        )
                        c0 += chk

                    # max over K=32: tree levels are contiguous block halves
                    l1 = tpool.tile([128, 16, 2, O], BF16)
                    nc.vector.tensor_max(
                        l1[:],
                        g2[:, 0:32, 0:O].rearrange("p (k b) e -> p k b e",
                                                   b=2),
                        g2[:, 32:64, 0:O].rearrange("p (k b) e -> p k b e",
                                                    b=2),
                    )
                    l2 = tpool.tile([128, 8, 2, O], BF16)
                    nc.vector.tensor_max(l2[:], l1[:, 0:8], l1[:, 8:16])
                    l3 = tpool.tile([128, 4, 2, O], BF16)
                    nc.vector.tensor_max(l3[:], l2[:, 0:4], l2[:, 4:8])
                    l4 = tpool.tile([128, 2, 2, O], BF16)
                    nc.vector.tensor_max(l4[:], l3[:, 0:2], l3[:, 2:4])
                    l5 = tpool.tile([128, 1, 2, O], BF16)
                    nc.vector.tensor_max(l5[:], l4[:, 0:1], l4[:, 1:2])

                    # transpose max to LSTM layout [ch, n]: two PE transposes
                    ps_f = psf.tile([64, NS], BF16)
                    nc.tensor.transpose(ps_f[:, 0:128], l5[:, 0, 0, :],
                                        ident[:])
                    nc.tensor.transpose(ps_f[:, 128:256], l5[:, 0, 1, :],
                                        ident[:])
                    mxT = fpool.tile([64, NS], BF16)
                    nc.scalar.activation(mxT[:], ps_f[:],
                                         mybir.ActivationFunctionType.Copy)
                    # u_self^T directly from a per-core matmul (x_my input)
                    ps_s = psf.tile([64, NS], F32)
                    nc.tensor.matmul(ps_s[:], cw[:], xm[:, a, :],
                                     start=True, stop=True)
                    usT = fpool.tile([64, NS], BF16)
                    nc.scalar.activation(usT[:], ps_s[:],
                                         mybir.ActivationFunctionType.Copy)
                    featT = fpool.tile([64, NS], BF16)
                    nc.vector.tensor_sub(featT[:], mxT[:], usT[:])

                    ps_g = psg.tile([64, 1024], F32)
                    for j in range(4):
                        nc.tensor.matmul(
                            ps_g[:, 256 * j:256 * (j + 1)],
                            wih[:, j, :],
                            featT[:],
                            start=True, stop=False,
                        )
                        nc.tensor.matmul(
                            ps_g[:, 256 * j:256 * (j + 1)],
                            whh[:, j, :],
                            h_aug[:],
                            start=False, stop=True,
                        )
                    # gate chunk order: i, f, o, g
                    sifo = lpool.tile([64, 768], BF16)
                    nc.scalar.activation(sifo[:], ps_g[:, 0:768],
                                         mybir.ActivationFunctionType.Sigmoid)
                    tg = lpool.tile([64, NS], BF16)
                    nc.scalar.activation(tg[:], ps_g[:, 768:1024],
                                         mybir.ActivationFunctionType.Tanh)
                    t1 = lpool.tile([64, NS], BF16)
                    nc.vector.tensor_mul(t1[:], sifo[:, 256:512], c_st[:])
                    t2 = lpool.tile([64, NS], BF16)
                    nc.vector.tensor_mul(t2[:], sifo[:, 0:256], tg[:])
                    nc.vector.tensor_add(c_st[:], t1[:], t2[:])
                    thc = lpool.tile([64, NS], BF16)
                    nc.scalar.activation(thc[:], c_st[:],
                                         mybir.ActivationFunctionType.Tanh)
                    nc.vector.tensor_mul(h_aug[0:64, :], sifo[:, 512:768],
                                         thc[:])
                    nc.sync.dma_start(oh_d.ap()[t], h_aug[0:64, :])

            nc.sync.dma_start(oc_d.ap()[:], c_st[:])

    nc.compile()
    return nc


def _prep_inputs(x, A, conv_w, conv_b, w_ih, w_hh, b_ih, b_hh):
    x = np.asarray(x, np.float32)
    A = np.asarray(A)
    conv_w = np.asarray(conv_w, np.float32)
    conv_b = np.asarray(conv_b, np.float32)
    w_ih = np.asarray(w_ih, np.float32)
    w_hh = np.asarray(w_hh, np.float32)
    b_ih = np.asarray(b_ih, np.float32)
    b_hh = np.asarray(b_hh, np.float32)

    xd = np.ascontiguousarray(
        x.transpose(1, 2, 0).reshape(PAIRS, 2, C, N).transpose(0, 2, 1, 3)
    ).astype(BF)                                            # [10, 32, 2, 2048]
    cw = np.ascontiguousarray(conv_w.T).astype(BF)          # [32, 64]

    # k-major flat index list [nbrs (k,n) | self], wrapped per 16 partitions
    # per chunk, replicated across the 8 GPSIMD cores' partition groups
    idx_all = np.empty((NCORES, 128, T, NIDX // 16), np.int16)
    ar = np.arange(NS, dtype=np.int16)
    for cid in range(NCORES):
        base = cid * NS
        for t in range(T):
            flat = np.ascontiguousarray(
                A[t, base:base + NS, :].T).reshape(-1).astype(np.int16)
            w = np.empty((16, NIDX // 16), np.int16)
            c0 = 0
            for chk in CHUNKS:
                w[:, c0 // 16:(c0 + chk) // 16] = \
                    flat[c0:c0 + chk].reshape(chk // 16, 16).T
                c0 += chk
            idx_all[cid, :, t, :] = np.tile(w, (8, 1))

    jorder = [0, 1, 3, 2]   # chunk order i,f,o,g from torch's i,f,g,o
    bias_eff = b_ih + b_hh + w_ih @ conv_b
    wih_l = np.empty((64, 4, 64), np.float32)
    whh_l = np.empty((65, 4, 64), np.float32)
    for j, jo in enumerate(jorder):
        wih_l[:, j, :] = w_ih[64 * jo:64 * (jo + 1), :].T
        whh_l[0:64, j, :] = w_hh[64 * jo:64 * (jo + 1), :].T
        whh_l[64, j, :] = bias_eff[64 * jo:64 * (jo + 1)]
    wih_l = wih_l.astype(BF)
    whh_l = whh_l.astype(BF)
    ident = np.eye(128, dtype=np.float32).astype(BF)

    in_maps = []
    for cid in range(NCORES):
        base = cid * NS
        in_maps.append({
            "xd": xd,
            "xm": np.ascontiguousarray(xd[:, :, :, base:base + NS]),
            "cw": cw,
            "idx": idx_all[cid],
            "wih": wih_l,
            "whh": whh_l,
            "ident": ident,
        })
    return in_maps


def kernel(x, A, conv_w, conv_b, w_ih, w_hh, b_ih, b_hh, _bench=None):
    if "nc" not in _CACHE:
        _CACHE["nc"] = _build()
    nc = _CACHE["nc"]

    in_maps = _prep_inputs(x, A, conv_w, conv_b, w_ih, w_hh, b_ih, b_hh)
    res = run_bass_kernel_spmd(nc, in_maps, core_ids=list(range(NCORES)),
                               **(_bench or {}))
    if _bench is not None:
        _CACHE["last_results"] = res

    out = np.empty((N, T, O), np.float32)
    cN = np.empty((N, 1, O), np.float32)
    for cid in range(NCORES):
        r = res.results[cid]
        base = cid * NS
        out[base:base + NS] = r["oh"].astype(np.float32).transpose(2, 0, 1)
        cN[base:base + NS, 0, :] = r["oc"].astype(np.float32).T
    hN = out[:, T - 1:T, :].copy()
    return out, hN, cN
